# revision 6
# baseline (speedup 1.0000x reference)
"""Trainium2 Bass kernel for nn_CustomLoss_71244917506189.

Math: the reference loss needs, per class j in {0, 1} (m_j = 1[target == j]):
    cnt_j   = sum_i m_j[i]
    ldsum_j = sum_i m_j[i] * logdet[i]
    Sm_j[d] = sum_i m_j[i] * mean[i, d]     -> mu_j = Sm_j / cnt_j
    Sl_j[d] = sum_i m_j[i] * log_sd[i, d]   -> lsd_j = Sl_j / cnt_j
    lp_j    = mean over class of sum_d(-LOG2PI/2 - lsd_j[d]
              - 0.5 * (z[i,d] - mu_j[d])^2 * exp(-2 lsd_j[d]))
Expanding the quadratic, sum_i m_j (z - mu_j)^2 w_j
    = sum_d w_j[d] * (Szz_j[d] - 2 mu_j[d] Sz_j[d] + cnt_j mu_j[d]^2)
so the whole computation collapses to four (2, D) masked batch-sums over
(z, z^2, mean, log_sd) plus two tiny (B,)-length reductions.  Each input
element is read from HBM exactly once -> memory-bound as intended.

Device strategy (8 cores, batch-sharded 1024 rows each):
    The masked sums are matmuls out[2, 512] = mask[128, 2]^T @ data[128, 512]
    accumulated in PSUM over the 8 batch-tiles of 128 rows.  Data is loaded
    with a casting SWDGE DMA straight into float32r tiles, which keeps the
    PE at 1 cycle/row (fp32 proper would be 4) with ~1e-4 relative error on
    the masked sums.  z^2 is produced by an ACT-engine Square.  Four
    bank-phases (z lo | z hi with z^2, mean, log_sd) keep PSUM within its
    8 banks.  Final combine across cores is a tiny (4, 2, 4096) host sum.
"""

import numpy as np

import concourse.bacc as bacc
import concourse.mybir as mybir
import concourse.tile as tile
from concourse.bass_utils import run_bass_kernel_spmd

B, D = 8192, 4096
N_CORES = 8
B_CORE = B // N_CORES  # 1024
P = 128                # partitions per batch-tile
T = B_CORE // P        # 8 batch-tiles per core
CHUNK = 512            # matmul moving free dim (one PSUM bank)
HALF = D // 2          # 2048
LOG2PI = float(np.log(2.0 * np.pi))

# test.py toggles this to capture an NTFF profile; the graded path leaves it
# off.  LAST_RESULTS holds the BassKernelResults of the most recent run.
TRACE = False
LAST_RESULTS = None

_NC_CACHE = None


def _build():
    nc = bacc.Bacc(
        "TRN2",
        target_bir_lowering=False,
        debug=False,
        num_devices=N_CORES,
        enable_asserts=False,
    )
    f32 = mybir.dt.float32
    f32r = mybir.dt.float32r
    z = nc.dram_tensor("z", (B_CORE, D), f32, kind="ExternalInput")
    mean = nc.dram_tensor("mean", (B_CORE, D), f32, kind="ExternalInput")
    lsd = nc.dram_tensor("lsd", (B_CORE, D), f32, kind="ExternalInput")
    # mask[p, 2*t + j] = 1.0 if target[t*128 + p] == j else 0.0
    mask = nc.dram_tensor("mask", (P, 2 * T), f32, kind="ExternalInput")
    # partial[0]=sum m_j z, [1]=sum m_j z^2, [2]=sum m_j mean, [3]=sum m_j lsd
    out = nc.dram_tensor("partial", (4, 2, D), f32, kind="ExternalOutput")

    with tile.TileContext(nc) as tc:
        with (
            tc.tile_pool(name="data", bufs=4) as data_pool,
            tc.tile_pool(name="sq", bufs=2) as sq_pool,
            tc.tile_pool(name="stage", bufs=2) as stage_pool,
            tc.tile_pool(name="maskp", bufs=1) as mask_pool,
            tc.tile_pool(name="acc", bufs=1, space="PSUM") as psum_pool,
        ):
            # 0/1 values are exact under the f32r rounding cast.
            mask_t = mask_pool.tile([P, 2 * T], f32r)
            nc.gpsimd.dma_start(out=mask_t, in_=mask[:, :])

            # Phases 0/1: z columns [half*2048, (half+1)*2048) and their
            # squares.  z chunks land in banks 0-3, z^2 chunks in banks 4-7.
            for half in range(2):
                acc = psum_pool.tile([2, D], f32, tag="acc")
                for t in range(T):
                    zt = data_pool.tile([P, HALF], f32r, tag="data")
                    nc.gpsimd.dma_start(
                        out=zt, in_=z[t * P : (t + 1) * P, half * HALF : (half + 1) * HALF]
                    )
                    zsq = sq_pool.tile([P, HALF], f32r, tag="sq")
                    nc.scalar.activation(
                        out=zsq, in_=zt, func=mybir.ActivationFunctionType.Square
                    )
                    lhsT = mask_t[:, 2 * t : 2 * t + 2]
                    for c in range(HALF // CHUNK):  # 4 chunks of 512
                        nc.tensor.matmul(
                            acc[:, c * CHUNK : (c + 1) * CHUNK],
                            lhsT=lhsT,
                            rhs=zt[:, c * CHUNK : (c + 1) * CHUNK],
                            start=(t == 0),
                            stop=(t == T - 1),
                        )
                        nc.tensor.matmul(
                            acc[:, (4 + c) * CHUNK : (5 + c) * CHUNK],
                            lhsT=lhsT,
                            rhs=zsq[:, c * CHUNK : (c + 1) * CHUNK],
                            start=(t == 0),
                            stop=(t == T - 1),
                        )
                st = stage_pool.tile([2, D], f32, tag="stage")
                nc.vector.tensor_copy(out=st, in_=acc)
                nc.sync.dma_start(
                    out=out[0, :, half * HALF : (half + 1) * HALF], in_=st[:, 0:HALF]
                )
                nc.sync.dma_start(
                    out=out[1, :, half * HALF : (half + 1) * HALF], in_=st[:, HALF:D]
                )

            # Phases 2/3: mean and log_sd, full 4096 columns -> banks 0-7.
            for idx, src in ((2, mean), (3, lsd)):
                acc = psum_pool.tile([2, D], f32, tag="acc")
                for t in range(T):
                    xt = data_pool.tile([P, D], f32r, tag="data")
                    nc.gpsimd.dma_start(out=xt, in_=src[t * P : (t + 1) * P, :])
                    lhsT = mask_t[:, 2 * t : 2 * t + 2]
                    for c in range(D // CHUNK):  # 8 chunks of 512
                        nc.tensor.matmul(
                            acc[:, c * CHUNK : (c + 1) * CHUNK],
                            lhsT=lhsT,
                            rhs=xt[:, c * CHUNK : (c + 1) * CHUNK],
                            start=(t == 0),
                            stop=(t == T - 1),
                        )
                st = stage_pool.tile([2, D], f32, tag="stage")
                nc.vector.tensor_copy(out=st, in_=acc)
                nc.sync.dma_start(out=out[idx], in_=st)

    nc.compile()
    return nc


def kernel(z, mean, log_sd, target, logdet):
    global _NC_CACHE, LAST_RESULTS
    z = np.ascontiguousarray(np.asarray(z, dtype=np.float32))
    mean = np.ascontiguousarray(np.asarray(mean, dtype=np.float32))
    log_sd = np.ascontiguousarray(np.asarray(log_sd, dtype=np.float32))
    target = np.asarray(target)
    logdet = np.asarray(logdet, dtype=np.float32)

    if _NC_CACHE is None:
        _NC_CACHE = _build()
    nc = _NC_CACHE

    # mask[p, 2t+j] = 1[target[core*1024 + t*128 + p] == j]
    tgt = target.reshape(N_CORES, T, P)
    in_maps = []
    for c in range(N_CORES):
        m = np.zeros((P, 2 * T), dtype=np.float32)
        for j in (0, 1):
            m[:, j::2] = (tgt[c].T == j).astype(np.float32)
        in_maps.append(
            {
                "z": z[c * B_CORE : (c + 1) * B_CORE],
                "mean": mean[c * B_CORE : (c + 1) * B_CORE],
                "lsd": log_sd[c * B_CORE : (c + 1) * B_CORE],
                "mask": m,
            }
        )

    res = run_bass_kernel_spmd(
        nc, in_maps, core_ids=list(range(N_CORES)), trace=TRACE
    )
    LAST_RESULTS = res

    # Cross-core reduction of the per-core partial sums, then the closed-form
    # finalization (all O(D) work).
    part = np.zeros((4, 2, D), dtype=np.float64)
    for c in range(N_CORES):
        part += res.results[c]["partial"].astype(np.float64)
    sz, szz, sm, sl = part  # each (2, D)

    cnt = np.array(
        [(target == 0).sum(), (target == 1).sum()], dtype=np.float64
    )
    ldsum = np.array(
        [
            logdet[target == 0].astype(np.float64).sum(),
            logdet[target == 1].astype(np.float64).sum(),
        ]
    )

    mu = sm / cnt[:, None]
    lsd_cls = sl / cnt[:, None]
    w = np.exp(-2.0 * lsd_cls)
    quad = (w * (szz - 2.0 * mu * sz + cnt[:, None] * mu * mu)).sum(axis=1)
    lp_mean = -0.5 * LOG2PI * D - lsd_cls.sum(axis=1) - 0.5 * quad / cnt
    ld_mean = ldsum / cnt
    prior_logprob = np.float32((lp_mean + ld_mean).mean())

    return (
        prior_logprob,
        mu.astype(np.float32),
        lsd_cls.astype(np.float32),
        lp_mean.astype(np.float32),
    )


# revision 7
# speedup vs baseline: 1.0814x; 1.0814x over previous
"""Trainium2 Bass kernel for nn_CustomLoss_71244917506189.

Math: the reference loss needs, per class j in {0, 1} (m_j = 1[target == j]):
    cnt_j   = sum_i m_j[i]
    ldsum_j = sum_i m_j[i] * logdet[i]
    Sm_j[d] = sum_i m_j[i] * mean[i, d]     -> mu_j = Sm_j / cnt_j
    Sl_j[d] = sum_i m_j[i] * log_sd[i, d]   -> lsd_j = Sl_j / cnt_j
    lp_j    = mean over class of sum_d(-LOG2PI/2 - lsd_j[d]
              - 0.5 * (z[i,d] - mu_j[d])^2 * exp(-2 lsd_j[d]))
Expanding the quadratic, sum_i m_j (z - mu_j)^2 w_j
    = sum_d w_j[d] * (Szz_j[d] - 2 mu_j[d] Sz_j[d] + cnt_j mu_j[d]^2)
so the whole computation collapses to four (2, D) masked batch-sums over
(z, z^2, mean, log_sd) plus two tiny (B,)-length reductions.  Each input
element is read from HBM exactly once -> memory-bound as intended.

Device strategy (8 cores, batch-sharded 1024 rows each):
    The masked sums are matmuls out[2, 512] = mask[128, 2]^T @ data[128, 512]
    accumulated in PSUM over the 8 batch-tiles of 128 rows.  Data is loaded
    with a casting SWDGE DMA straight into float32r tiles, which keeps the
    PE at 1 cycle/row (fp32 proper would be 4) with ~1e-4 relative error on
    the masked sums.  z^2 is produced by an ACT-engine Square.  Four
    bank-phases (z lo | z hi with z^2, mean, log_sd) keep PSUM within its
    8 banks.  Final combine across cores is a tiny (4, 2, 4096) host sum.
"""

import numpy as np

import concourse.bacc as bacc
import concourse.mybir as mybir
import concourse.tile as tile
from concourse.bass_utils import run_bass_kernel_spmd

B, D = 8192, 4096
N_CORES = 8
B_CORE = B // N_CORES  # 1024
P = 128                # partitions per batch-tile
T = B_CORE // P        # 8 batch-tiles per core
CHUNK = 512            # matmul moving free dim (one PSUM bank)
HALF = D // 2          # 2048
LOG2PI = float(np.log(2.0 * np.pi))

# test.py toggles this to capture an NTFF profile; the graded path leaves it
# off.  LAST_RESULTS holds the BassKernelResults of the most recent run.
TRACE = False
LAST_RESULTS = None

_NC_CACHE = None


def _build():
    nc = bacc.Bacc(
        "TRN2",
        target_bir_lowering=False,
        debug=False,
        num_devices=N_CORES,
        enable_asserts=False,
    )
    f32 = mybir.dt.float32
    f32r = mybir.dt.float32r
    z = nc.dram_tensor("z", (B_CORE, D), f32, kind="ExternalInput")
    mean = nc.dram_tensor("mean", (B_CORE, D), f32, kind="ExternalInput")
    lsd = nc.dram_tensor("lsd", (B_CORE, D), f32, kind="ExternalInput")
    # mask[p, 2*t + j] = 1.0 if target[t*128 + p] == j else 0.0
    mask = nc.dram_tensor("mask", (P, 2 * T), f32, kind="ExternalInput")
    # partial[0]=sum m_j z, [1]=sum m_j z^2, [2]=sum m_j mean, [3]=sum m_j lsd
    out = nc.dram_tensor("partial", (4, 2, D), f32, kind="ExternalOutput")

    with tile.TileContext(nc) as tc:
        with (
            tc.tile_pool(name="zdata", bufs=3) as zdata_pool,
            tc.tile_pool(name="mdata", bufs=3) as mdata_pool,
            tc.tile_pool(name="sq", bufs=2) as sq_pool,
            tc.tile_pool(name="stage", bufs=2) as stage_pool,
            tc.tile_pool(name="maskp", bufs=1) as mask_pool,
            tc.tile_pool(name="acc", bufs=1, space="PSUM") as psum_pool,
        ):
            # 0/1 values are exact under the f32r rounding cast.
            mask_t = mask_pool.tile([P, 2 * T], f32r)
            nc.gpsimd.dma_start(out=mask_t, in_=mask[:, :])

            # z viewed as [pair tt, sub-tile j, partition p, column d] so two
            # 128-row batch-tiles arrive in one 2 MB DMA.
            z4 = z.rearrange("(tt j p) d -> tt j p d", j=2, p=P)

            def drain(acc, st):
                # Split the PSUM->SBUF eviction across DVE and ACT so the
                # next phase's WAR stall on the PSUM banks is halved.
                nc.vector.tensor_copy(out=st[:, 0:HALF], in_=acc[:, 0:HALF])
                nc.scalar.copy(out=st[:, HALF:D], in_=acc[:, HALF:D])

            # Phases 0/1: z columns [half*2048, (half+1)*2048) and their
            # squares.  z chunks land in banks 0-3, z^2 chunks in banks 4-7.
            for half in range(2):
                acc = psum_pool.tile([2, D], f32, tag="acc")
                for tt in range(T // 2):
                    zt = zdata_pool.tile([P, 2, HALF], f32r, tag="zdata")
                    nc.gpsimd.dma_start(
                        out=zt,
                        in_=z4[tt, :, :, half * HALF : (half + 1) * HALF].rearrange(
                            "j p d -> p j d"
                        ),
                    )
                    for j in range(2):
                        t = 2 * tt + j
                        zsq = sq_pool.tile([P, HALF], f32r, tag="sq")
                        nc.scalar.activation(
                            out=zsq,
                            in_=zt[:, j, :],
                            func=mybir.ActivationFunctionType.Square,
                        )
                        lhsT = mask_t[:, 2 * t : 2 * t + 2]
                        for c in range(HALF // CHUNK):  # 4 chunks of 512
                            nc.tensor.matmul(
                                acc[:, c * CHUNK : (c + 1) * CHUNK],
                                lhsT=lhsT,
                                rhs=zt[:, j, c * CHUNK : (c + 1) * CHUNK],
                                start=(t == 0),
                                stop=(t == T - 1),
                            )
                            nc.tensor.matmul(
                                acc[:, (4 + c) * CHUNK : (5 + c) * CHUNK],
                                lhsT=lhsT,
                                rhs=zsq[:, c * CHUNK : (c + 1) * CHUNK],
                                start=(t == 0),
                                stop=(t == T - 1),
                            )
                st = stage_pool.tile([2, D], f32, tag="stage")
                drain(acc, st)
                nc.sync.dma_start(
                    out=out[0, :, half * HALF : (half + 1) * HALF], in_=st[:, 0:HALF]
                )
                nc.sync.dma_start(
                    out=out[1, :, half * HALF : (half + 1) * HALF], in_=st[:, HALF:D]
                )

            # Phases 2/3: mean and log_sd, full 4096 columns -> banks 0-7.
            for idx, src in ((2, mean), (3, lsd)):
                acc = psum_pool.tile([2, D], f32, tag="acc")
                for t in range(T):
                    xt = mdata_pool.tile([P, D], f32r, tag="mdata")
                    nc.gpsimd.dma_start(out=xt, in_=src[t * P : (t + 1) * P, :])
                    lhsT = mask_t[:, 2 * t : 2 * t + 2]
                    for c in range(D // CHUNK):  # 8 chunks of 512
                        nc.tensor.matmul(
                            acc[:, c * CHUNK : (c + 1) * CHUNK],
                            lhsT=lhsT,
                            rhs=xt[:, c * CHUNK : (c + 1) * CHUNK],
                            start=(t == 0),
                            stop=(t == T - 1),
                        )
                st = stage_pool.tile([2, D], f32, tag="stage")
                drain(acc, st)
                nc.sync.dma_start(out=out[idx], in_=st)

    nc.compile()
    return nc


def kernel(z, mean, log_sd, target, logdet):
    global _NC_CACHE, LAST_RESULTS
    z = np.ascontiguousarray(np.asarray(z, dtype=np.float32))
    mean = np.ascontiguousarray(np.asarray(mean, dtype=np.float32))
    log_sd = np.ascontiguousarray(np.asarray(log_sd, dtype=np.float32))
    target = np.asarray(target)
    logdet = np.asarray(logdet, dtype=np.float32)

    if _NC_CACHE is None:
        _NC_CACHE = _build()
    nc = _NC_CACHE

    # mask[p, 2t+j] = 1[target[core*1024 + t*128 + p] == j]
    tgt = target.reshape(N_CORES, T, P)
    in_maps = []
    for c in range(N_CORES):
        m = np.zeros((P, 2 * T), dtype=np.float32)
        for j in (0, 1):
            m[:, j::2] = (tgt[c].T == j).astype(np.float32)
        in_maps.append(
            {
                "z": z[c * B_CORE : (c + 1) * B_CORE],
                "mean": mean[c * B_CORE : (c + 1) * B_CORE],
                "lsd": log_sd[c * B_CORE : (c + 1) * B_CORE],
                "mask": m,
            }
        )

    res = run_bass_kernel_spmd(
        nc, in_maps, core_ids=list(range(N_CORES)), trace=TRACE
    )
    LAST_RESULTS = res

    # Cross-core reduction of the per-core partial sums, then the closed-form
    # finalization (all O(D) work).
    part = np.zeros((4, 2, D), dtype=np.float64)
    for c in range(N_CORES):
        part += res.results[c]["partial"].astype(np.float64)
    sz, szz, sm, sl = part  # each (2, D)

    cnt = np.array(
        [(target == 0).sum(), (target == 1).sum()], dtype=np.float64
    )
    ldsum = np.array(
        [
            logdet[target == 0].astype(np.float64).sum(),
            logdet[target == 1].astype(np.float64).sum(),
        ]
    )

    mu = sm / cnt[:, None]
    lsd_cls = sl / cnt[:, None]
    w = np.exp(-2.0 * lsd_cls)
    quad = (w * (szz - 2.0 * mu * sz + cnt[:, None] * mu * mu)).sum(axis=1)
    lp_mean = -0.5 * LOG2PI * D - lsd_cls.sum(axis=1) - 0.5 * quad / cnt
    ld_mean = ldsum / cnt
    prior_logprob = np.float32((lp_mean + ld_mean).mean())

    return (
        prior_logprob,
        mu.astype(np.float32),
        lsd_cls.astype(np.float32),
        lp_mean.astype(np.float32),
    )


# revision 10
# speedup vs baseline: 1.1777x; 1.0891x over previous
"""Trainium2 Bass kernel for nn_CustomLoss_71244917506189.

Math: the reference loss needs, per class j in {0, 1} (m_j = 1[target == j]):
    cnt_j   = sum_i m_j[i]
    ldsum_j = sum_i m_j[i] * logdet[i]
    Sm_j[d] = sum_i m_j[i] * mean[i, d]     -> mu_j = Sm_j / cnt_j
    Sl_j[d] = sum_i m_j[i] * log_sd[i, d]   -> lsd_j = Sl_j / cnt_j
    lp_j    = mean over class of sum_d(-LOG2PI/2 - lsd_j[d]
              - 0.5 * (z[i,d] - mu_j[d])^2 * exp(-2 lsd_j[d]))
Expanding the quadratic, sum_i m_j (z - mu_j)^2 w_j
    = sum_d w_j[d] * (Szz_j[d] - 2 mu_j[d] Sz_j[d] + cnt_j mu_j[d]^2)
so the whole computation collapses to four (2, D) masked batch-sums over
(z, z^2, mean, log_sd) plus two tiny (B,)-length reductions.  Each input
element is read from HBM exactly once -> memory-bound as intended.

Device strategy (8 cores, batch-sharded 1024 rows each):
    The masked sums are matmuls out[2, 512] = mask[128, 2]^T @ data[128, 512]
    accumulated in PSUM over the 8 batch-tiles of 128 rows.  Data is loaded
    with a casting SWDGE DMA straight into float32r tiles, which keeps the
    PE at 1 cycle/row (fp32 proper would be 4) with ~1e-4 relative error on
    the masked sums.  z^2 is produced by an ACT-engine Square.  Four
    bank-phases (z lo | z hi with z^2, mean, log_sd) keep PSUM within its
    8 banks.  Final combine across cores is a tiny (4, 2, 4096) host sum.
"""

import numpy as np

import concourse.bacc as bacc
import concourse.mybir as mybir
import concourse.tile as tile
from concourse.bass_utils import run_bass_kernel_spmd

B, D = 8192, 4096
N_CORES = 8
B_CORE = B // N_CORES  # 1024
P = 128                # partitions per batch-tile
T = B_CORE // P        # 8 batch-tiles per core
CHUNK = 512            # matmul moving free dim (one PSUM bank)
HALF = D // 2          # 2048
LOG2PI = float(np.log(2.0 * np.pi))

# test.py toggles this to capture an NTFF profile; the graded path leaves it
# off.  LAST_RESULTS holds the BassKernelResults of the most recent run.
TRACE = False
LAST_RESULTS = None

_NC_CACHE = None


def _build():
    nc = bacc.Bacc(
        "TRN2",
        target_bir_lowering=False,
        debug=False,
        num_devices=N_CORES,
        enable_asserts=False,
    )
    f32 = mybir.dt.float32
    f32r = mybir.dt.float32r
    z = nc.dram_tensor("z", (B_CORE, D), f32, kind="ExternalInput")
    mean = nc.dram_tensor("mean", (B_CORE, D), f32, kind="ExternalInput")
    lsd = nc.dram_tensor("lsd", (B_CORE, D), f32, kind="ExternalInput")
    # mask[p, 2*t + j] = 1.0 if target[t*128 + p] == j else 0.0
    mask = nc.dram_tensor("mask", (P, 2 * T), f32, kind="ExternalInput")
    # partial[0]=sum m_j z, [1]=sum m_j z^2, [2]=sum m_j mean, [3]=sum m_j lsd
    out = nc.dram_tensor("partial", (4, 2, D), f32, kind="ExternalOutput")

    from concourse.bass import _add_dep_helper

    # The mask weights stay stationary in the PE across all chunk matmuls of
    # a batch-tile: only the first matmul self-loads (fp32r forbids a
    # standalone LDWEIGHTS), the rest set ldweights=False.  That makes PE
    # program order semantic, so chain every matmul to its predecessor.
    _prev_mm = [None]

    def mm(out_ap, lhsT, rhs, start, stop, fresh_weights):
        inst = nc.tensor.matmul(out_ap, lhsT=lhsT, rhs=rhs, start=start, stop=stop)
        if not fresh_weights:
            inst.ins.ldweights = False
        if _prev_mm[0] is not None:
            _add_dep_helper(
                inst.ins, _prev_mm[0].ins, sync=False, reason="stationary-weight order"
            )
        _prev_mm[0] = inst
        return inst

    with tile.TileContext(nc) as tc:
        with (
            tc.tile_pool(name="zdata", bufs=3) as zdata_pool,
            tc.tile_pool(name="mdata", bufs=3) as mdata_pool,
            tc.tile_pool(name="sq", bufs=2) as sq_pool,
            tc.tile_pool(name="stage", bufs=2) as stage_pool,
            tc.tile_pool(name="maskp", bufs=1) as mask_pool,
            tc.tile_pool(name="acc", bufs=1, space="PSUM") as psum_pool,
        ):
            # 0/1 values are exact under the f32r rounding cast.
            mask_t = mask_pool.tile([P, 2 * T], f32r)
            nc.gpsimd.dma_start(out=mask_t, in_=mask[:, :])

            # z viewed as [pair tt, sub-tile j, partition p, column d] so two
            # 128-row batch-tiles arrive in one 2 MB DMA.
            z4 = z.rearrange("(tt j p) d -> tt j p d", j=2, p=P)

            def drain(acc, st):
                # Split the PSUM->SBUF eviction across DVE and ACT so the
                # next phase's WAR stall on the PSUM banks is halved.
                nc.vector.tensor_copy(out=st[:, 0:HALF], in_=acc[:, 0:HALF])
                nc.scalar.copy(out=st[:, HALF:D], in_=acc[:, HALF:D])

            # Phases 0/1: z columns [half*2048, (half+1)*2048) and their
            # squares.  z chunks land in banks 0-3, z^2 chunks in banks 4-7.
            for half in range(2):
                acc = psum_pool.tile([2, D], f32, tag="acc")
                for tt in range(T // 2):
                    zt = zdata_pool.tile([P, 2, HALF], f32r, tag="zdata")
                    nc.gpsimd.dma_start(
                        out=zt,
                        in_=z4[tt, :, :, half * HALF : (half + 1) * HALF].rearrange(
                            "j p d -> p j d"
                        ),
                    )
                    for j in range(2):
                        t = 2 * tt + j
                        zsq = sq_pool.tile([P, HALF], f32r, tag="sq")
                        nc.scalar.activation(
                            out=zsq,
                            in_=zt[:, j, :],
                            func=mybir.ActivationFunctionType.Square,
                        )
                        lhsT = mask_t[:, 2 * t : 2 * t + 2]
                        for c in range(HALF // CHUNK):  # 4 chunks of 512
                            mm(
                                acc[:, c * CHUNK : (c + 1) * CHUNK],
                                lhsT,
                                zt[:, j, c * CHUNK : (c + 1) * CHUNK],
                                start=(t == 0),
                                stop=(t == T - 1),
                                fresh_weights=(c == 0),
                            )
                            mm(
                                acc[:, (4 + c) * CHUNK : (5 + c) * CHUNK],
                                lhsT,
                                zsq[:, c * CHUNK : (c + 1) * CHUNK],
                                start=(t == 0),
                                stop=(t == T - 1),
                                fresh_weights=False,
                            )
                st = stage_pool.tile([2, D], f32, tag="stage")
                drain(acc, st)
                nc.sync.dma_start(
                    out=out[0, :, half * HALF : (half + 1) * HALF], in_=st[:, 0:HALF]
                )
                nc.sync.dma_start(
                    out=out[1, :, half * HALF : (half + 1) * HALF], in_=st[:, HALF:D]
                )

            # Phases 2/3: mean and log_sd, full 4096 columns -> banks 0-7.
            for idx, src in ((2, mean), (3, lsd)):
                acc = psum_pool.tile([2, D], f32, tag="acc")
                for t in range(T):
                    xt = mdata_pool.tile([P, D], f32r, tag="mdata")
                    nc.gpsimd.dma_start(out=xt, in_=src[t * P : (t + 1) * P, :])
                    lhsT = mask_t[:, 2 * t : 2 * t + 2]
                    for c in range(D // CHUNK):  # 8 chunks of 512
                        mm(
                            acc[:, c * CHUNK : (c + 1) * CHUNK],
                            lhsT,
                            xt[:, c * CHUNK : (c + 1) * CHUNK],
                            start=(t == 0),
                            stop=(t == T - 1),
                            fresh_weights=(c == 0),
                        )
                st = stage_pool.tile([2, D], f32, tag="stage")
                drain(acc, st)
                nc.sync.dma_start(out=out[idx], in_=st)

    nc.compile()
    return nc


def kernel(z, mean, log_sd, target, logdet):
    global _NC_CACHE, LAST_RESULTS
    z = np.ascontiguousarray(np.asarray(z, dtype=np.float32))
    mean = np.ascontiguousarray(np.asarray(mean, dtype=np.float32))
    log_sd = np.ascontiguousarray(np.asarray(log_sd, dtype=np.float32))
    target = np.asarray(target)
    logdet = np.asarray(logdet, dtype=np.float32)

    if _NC_CACHE is None:
        _NC_CACHE = _build()
    nc = _NC_CACHE

    # mask[p, 2t+j] = 1[target[core*1024 + t*128 + p] == j]
    tgt = target.reshape(N_CORES, T, P)
    in_maps = []
    for c in range(N_CORES):
        m = np.zeros((P, 2 * T), dtype=np.float32)
        for j in (0, 1):
            m[:, j::2] = (tgt[c].T == j).astype(np.float32)
        in_maps.append(
            {
                "z": z[c * B_CORE : (c + 1) * B_CORE],
                "mean": mean[c * B_CORE : (c + 1) * B_CORE],
                "lsd": log_sd[c * B_CORE : (c + 1) * B_CORE],
                "mask": m,
            }
        )

    res = run_bass_kernel_spmd(
        nc, in_maps, core_ids=list(range(N_CORES)), trace=TRACE
    )
    LAST_RESULTS = res

    # Cross-core reduction of the per-core partial sums, then the closed-form
    # finalization (all O(D) work).
    part = np.zeros((4, 2, D), dtype=np.float64)
    for c in range(N_CORES):
        part += res.results[c]["partial"].astype(np.float64)
    sz, szz, sm, sl = part  # each (2, D)

    cnt = np.array(
        [(target == 0).sum(), (target == 1).sum()], dtype=np.float64
    )
    ldsum = np.array(
        [
            logdet[target == 0].astype(np.float64).sum(),
            logdet[target == 1].astype(np.float64).sum(),
        ]
    )

    mu = sm / cnt[:, None]
    lsd_cls = sl / cnt[:, None]
    w = np.exp(-2.0 * lsd_cls)
    quad = (w * (szz - 2.0 * mu * sz + cnt[:, None] * mu * mu)).sum(axis=1)
    lp_mean = -0.5 * LOG2PI * D - lsd_cls.sum(axis=1) - 0.5 * quad / cnt
    ld_mean = ldsum / cnt
    prior_logprob = np.float32((lp_mean + ld_mean).mean())

    return (
        prior_logprob,
        mu.astype(np.float32),
        lsd_cls.astype(np.float32),
        lp_mean.astype(np.float32),
    )
